# revision 9
# baseline (speedup 1.0000x reference)
"""AFT-Full attention kernel for 8 TRN2 NeuronCores.

Math: the reference's exp_pos_bias = exp(pos_bias - max(pos_bias, axis=0)) is
identically 1.0 (the max is over a singleton dim), so the two (b,Ti,Tj,Dh)
einsums collapse to per-(b,h) sums over j:
    num[b,h] = sum_j exp(K-max_b K)[b,j,h] * V[b,j,h]
    den[b,h] = sum_j exp(K-max_b K)[b,j,h]
    out = (sigmoid(Q) * num/den) @ Wo.T

Sharding: sequence-parallel over T (256 positions per core, all 4 batches),
weights replicated.  Per-core row index r = b*256 + t_local (b-major) so the
per-(b,h) sums over t are contiguous innermost reduces.  One 32 KiB AllReduce
combines the per-core partial num/den; a 4-byte dummy AllReduce issued at
kernel start absorbs the ~60us one-time collective channel setup so the real
one (~12us) hides behind the Q projection.
"""

import numpy as np
import ml_dtypes

import concourse.bass as bass
import concourse.mybir as mybir
import concourse.tile as tile
from concourse import bacc
from concourse.bass_utils import run_bass_kernel_spmd

B, T, DM, DH = 4, 2048, 1024, 1024
N_CORES = 8
TC = T // N_CORES          # 256 sequence positions per core
R = B * TC                 # 1024 rows per core, r = b*256 + t
P = 128
MC = DM // P               # 8 contraction chunks (d_model)
HB = DH // P               # 8 hidden blocks
NB = 512                   # matmul moving free dim
RB = R // NB               # 2 row blocks
MB = DM // NB              # 2 output-model blocks

F16 = mybir.dt.float16
F32 = mybir.dt.float32
NPF16 = np.float16

_GRAPH = None


def _body(nc, tc):
    qT = nc.dram_tensor("qT", [DM, R], F16, kind="ExternalInput").ap()
    kT = nc.dram_tensor("kT", [DM, R], F16, kind="ExternalInput").ap()
    vT = nc.dram_tensor("vT", [DM, R], F16, kind="ExternalInput").ap()
    wqT = nc.dram_tensor("wqT", [DM, DH], F16, kind="ExternalInput").ap()
    wkT = nc.dram_tensor("wkT", [DM, DH], F16, kind="ExternalInput").ap()
    wvT = nc.dram_tensor("wvT", [DM, DH], F16, kind="ExternalInput").ap()
    woT = nc.dram_tensor("woT", [DH, DM], F16, kind="ExternalInput").ap()
    out = nc.dram_tensor("out", [R, DM], F32, kind="ExternalOutput").ap()

    Exp = mybir.ActivationFunctionType.Exp
    Sigmoid = mybir.ActivationFunctionType.Sigmoid
    Op = mybir.AluOpType

    from contextlib import ExitStack
    with ExitStack() as ctx:
        acts = ctx.enter_context(tc.tile_pool(name="acts", bufs=1))
        work = ctx.enter_context(tc.tile_pool(name="work", bufs=2))
        sqp = ctx.enter_context(tc.tile_pool(name="sqp", bufs=1))
        psum = ctx.enter_context(tc.tile_pool(name="psum", bufs=2, space="PSUM"))
        dram = ctx.enter_context(tc.tile_pool(name="dram", bufs=1, space="DRAM"))

        # Dummy 4-byte AllReduce issued first: the first collective doorbell
        # pays a ~60us one-time channel setup; absorbing it here (concurrent
        # with the DMA ramp and K/V phase) makes the real AllReduce ~12us.
        dummy = acts.tile([1, 1], F32, name="dummy")
        nc.gpsimd.memset(dummy[:], 0.0)
        d_in = dram.tile([1, 1], F32, name="d_in")
        d_out = dram.tile([1, 1], F32, name="d_out", addr_space="Shared")
        nc.sync.dma_start(d_in[:], dummy[:])
        nc.gpsimd.collective_compute(
            "AllReduce", mybir.AluOpType.add,
            replica_groups=[list(range(N_CORES))],
            ins=[d_in.opt()], outs=[d_out.opt()],
        )

        def declare(name, free):
            return acts.tile([P, MC, free], F16, name=name)

        def load_chunk(t, ap_dram, mc):
            src = ap_dram.rearrange("(c p) f -> p c f", p=P)
            nc.sync.dma_start(t[:, mc, :], src[:, mc, :])

        def load(ap_dram, name, free):
            t = declare(name, free)
            nc.sync.dma_start(t[:], ap_dram.rearrange("(c p) f -> p c f", p=P))
            return t

        # K/V operands stream in per-mc so the first matmuls start early
        # (sync ring drains in order).
        kt = declare("kt", R)
        wk = declare("wk", DH)
        for mc in range(MC):
            load_chunk(wk, wkT, mc)
            load_chunk(kt, kT, mc)
        vt = declare("vt", R)
        wv = declare("wv", DH)
        for mc in range(MC):
            load_chunk(wv, wvT, mc)
            load_chunk(vt, vT, mc)
        qt = load(qT, "qt", R)
        wq = load(wqT, "wq", DH)
        wo = load(woT, "wo", DM)

        nd_all = acts.tile([P, HB, 8], F32, name="nd_all")

        # ---- K/V projections + exp + partial num/den ----
        for hb in range(HB):
            hs = slice(hb * P, (hb + 1) * P)
            pk = psum.tile([P, R], F32, name="pk", tag="pk")
            for mc in range(MC):
                for rb in range(RB):
                    nc.tensor.matmul(
                        pk[:, rb * NB:(rb + 1) * NB],
                        wk[:, mc, hs],
                        kt[:, mc, rb * NB:(rb + 1) * NB],
                        start=(mc == 0), stop=(mc == MC - 1),
                    )
            pv = psum.tile([P, R], F32, name="pv", tag="pv")
            for mc in range(MC):
                for rb in range(RB):
                    nc.tensor.matmul(
                        pv[:, rb * NB:(rb + 1) * NB],
                        wv[:, mc, hs],
                        vt[:, mc, rb * NB:(rb + 1) * NB],
                        start=(mc == 0), stop=(mc == MC - 1),
                    )
            # max over batch: copy + 3 maxes (DVE may read PSUM only once/op)
            mk = work.tile([P, TC], F32, name="mk")
            nc.vector.tensor_copy(mk[:], pk[:, 0 * TC:1 * TC])
            for b in range(1, B):
                nc.vector.tensor_tensor(
                    mk[:], mk[:], pk[:, b * TC:(b + 1) * TC], op=Op.max)
            ek = work.tile([P, R], F32, name="ek")
            ek3 = ek.rearrange("p (b t) -> p b t", t=TC)
            nc.vector.tensor_tensor(
                ek3, pk.rearrange("p (b t) -> p b t", t=TC),
                mk[:, None, :].to_broadcast((P, B, TC)), op=Op.subtract)
            # exp on ACT with fused per-b den accumulation (contiguous slices)
            for b in range(B):
                bs = slice(b * TC, (b + 1) * TC)
                nc.scalar.activation(
                    ek[:, bs], ek[:, bs], Exp,
                    accum_out=nd_all[:, hb, b:b + 1])
            ekv = work.tile([P, R], F32, name="ekv")
            nc.vector.tensor_tensor(ekv[:], ek[:], pv[:], op=Op.mult)
            nc.vector.tensor_reduce(
                nd_all[:, hb, B:2 * B], ekv.rearrange("p (b t) -> p b t", t=TC),
                axis=mybir.AxisListType.X, op=Op.add)

        # ---- AllReduce of partial num/den (32 KiB) ----
        red_in = dram.tile([P, HB * 8], F32, name="red_in")
        red_out = dram.tile([P, HB * 8], F32, name="red_out", addr_space="Shared")
        nc.sync.dma_start(red_in[:], nd_all[:])
        nc.gpsimd.collective_compute(
            "AllReduce", Op.add,
            replica_groups=[list(range(N_CORES))],
            ins=[red_in.opt()], outs=[red_out.opt()],
        )
        nd_sum = acts.tile([P, HB, 8], F32, name="nd_sum")
        nc.sync.dma_start(nd_sum[:], red_out[:])

        # ---- Q projection + sigmoid (overlaps the collective) ----
        sq = []
        for hb in range(HB):
            hs = slice(hb * P, (hb + 1) * P)
            pq = psum.tile([P, R], F32, name="pq", tag="pk")
            for mc in range(MC):
                for rb in range(RB):
                    nc.tensor.matmul(
                        pq[:, rb * NB:(rb + 1) * NB],
                        wq[:, mc, hs],
                        qt[:, mc, rb * NB:(rb + 1) * NB],
                        start=(mc == 0), stop=(mc == MC - 1),
                    )
            s = sqp.tile([P, R], F16, name=f"sq{hb}")
            nc.scalar.activation(s[:], pq[:], Sigmoid)
            sq.append(s)

        # ---- r = num/den, yt = sigmoid(Q) * r ----
        rden = acts.tile([P, HB, B], F32, name="rden")
        nc.vector.reciprocal(rden[:], nd_sum[:, :, 0:B])
        r_bf = acts.tile([P, HB, B], F16, name="r_bf")
        nc.vector.tensor_tensor(r_bf[:], nd_sum[:, :, B:2 * B], rden[:], op=Op.mult)
        for hb in range(HB):
            s3 = sq[hb].rearrange("p (b t) -> p b t", t=TC)
            nc.vector.tensor_tensor(
                s3, s3, r_bf[:, hb, :, None].to_broadcast((P, B, TC)), op=Op.mult)

        # ---- output projection ----
        for rblk in range(HB):
            rs = slice(rblk * P, (rblk + 1) * P)
            po = psum.tile([P, DM], F32, name="po", tag="pv")
            for hc in range(HB):
                for mb in range(MB):
                    nc.tensor.matmul(
                        po[:, mb * NB:(mb + 1) * NB],
                        sq[hc][:, rs],
                        wo[:, hc, mb * NB:(mb + 1) * NB],
                        start=(hc == 0), stop=(hc == HB - 1),
                    )
            ot = work.tile([P, DM], F32, name="ot")
            if rblk % 2 == 0:
                nc.vector.tensor_copy(ot[:], po[:])
            else:
                nc.scalar.copy(ot[:], po[:])
            nc.sync.dma_start(out[rs, :], ot[:])


def _build():
    global _GRAPH
    if _GRAPH is None:
        nc = bacc.Bacc("TRN2", target_bir_lowering=False, debug=False,
                       num_devices=N_CORES)
        with tile.TileContext(nc) as tc:
            _body(nc, tc)
        nc.compile()
        _GRAPH = nc
    return _GRAPH


def _shard_inputs(inputs):
    q = np.asarray(inputs["q"], np.float32)
    k = np.asarray(inputs["k"], np.float32)
    v = np.asarray(inputs["v"], np.float32)
    wqT = np.ascontiguousarray(np.asarray(inputs["Wq"], np.float32).T).astype(NPF16)
    wkT = np.ascontiguousarray(np.asarray(inputs["Wk"], np.float32).T).astype(NPF16)
    wvT = np.ascontiguousarray(np.asarray(inputs["Wv"], np.float32).T).astype(NPF16)
    woT = np.ascontiguousarray(np.asarray(inputs["Wo"], np.float32).T).astype(NPF16)

    def tslice(x, c):
        # (B, TC, DM) -> (DM, B, TC) -> (DM, R) with r = b*256 + t
        s = x[:, c * TC:(c + 1) * TC, :].transpose(2, 0, 1)
        return np.ascontiguousarray(s).reshape(DM, R).astype(NPF16)

    in_maps = []
    for c in range(N_CORES):
        in_maps.append({
            "qT": tslice(q, c),
            "kT": tslice(k, c),
            "vT": tslice(v, c),
            "wqT": wqT, "wkT": wkT, "wvT": wvT, "woT": woT,
        })
    return in_maps


def _unshard(outs):
    full = np.empty((B, T, DM), np.float32)
    for c in range(N_CORES):
        # out_c[r, m] with r = b*256 + t  ->  (b, t, m)
        full[:, c * TC:(c + 1) * TC, :] = outs[c].reshape(B, TC, DM)
    return full


def run(inputs, trace=False, trace_cores=None, **kw):
    nc = _build()
    in_maps = _shard_inputs(inputs)
    res = run_bass_kernel_spmd(
        nc, in_maps, list(range(N_CORES)),
        trace=trace, trace_cores=trace_cores, **kw)
    return _unshard([m["out"] for m in res.results]), res


def kernel(**inputs):
    out, _ = run(inputs)
    return out


# revision 12
# speedup vs baseline: 1.0383x; 1.0383x over previous
"""AFT-Full attention kernel for 8 TRN2 NeuronCores.

Math: the reference's exp_pos_bias = exp(pos_bias - max(pos_bias, axis=0)) is
identically 1.0 (the max is over a singleton dim), so the two (b,Ti,Tj,Dh)
einsums collapse to per-(b,h) sums over j:
    num[b,h] = sum_j exp(K-max_b K)[b,j,h] * V[b,j,h]
    den[b,h] = sum_j exp(K-max_b K)[b,j,h]
    out = (sigmoid(Q) * num/den) @ Wo.T

Sharding: sequence-parallel over T (256 positions per core, all 4 batches),
weights replicated.  Per-core row index r = b*256 + t_local (b-major) so the
per-(b,h) sums over t are contiguous innermost reduces.  One 32 KiB AllReduce
combines the per-core partial num/den; a 4-byte dummy AllReduce issued at
kernel start absorbs the ~60us one-time collective channel setup so the real
one (~12us) hides behind the Q projection.
"""

import numpy as np
import ml_dtypes

import concourse.bass as bass
import concourse.mybir as mybir
import concourse.tile as tile
from concourse import bacc
from concourse.bass_utils import run_bass_kernel_spmd

B, T, DM, DH = 4, 2048, 1024, 1024
N_CORES = 8
TC = T // N_CORES          # 256 sequence positions per core
R = B * TC                 # 1024 rows per core, r = b*256 + t
P = 128
MC = DM // P               # 8 contraction chunks (d_model)
HB = DH // P               # 8 hidden blocks
NB = 512                   # matmul moving free dim
RB = R // NB               # 2 row blocks
MB = DM // NB              # 2 output-model blocks

F16 = mybir.dt.float16
F32 = mybir.dt.float32
NPF16 = np.float16

_GRAPH = None


def _body(nc, tc):
    qT = nc.dram_tensor("qT", [DM, R], F16, kind="ExternalInput").ap()
    kT = nc.dram_tensor("kT", [DM, R], F16, kind="ExternalInput").ap()
    vT = nc.dram_tensor("vT", [DM, R], F16, kind="ExternalInput").ap()
    wqT = nc.dram_tensor("wqT", [DM, DH], F16, kind="ExternalInput").ap()
    wkT = nc.dram_tensor("wkT", [DM, DH], F16, kind="ExternalInput").ap()
    wvT = nc.dram_tensor("wvT", [DM, DH], F16, kind="ExternalInput").ap()
    woT = nc.dram_tensor("woT", [DH, DM], F16, kind="ExternalInput").ap()
    out = nc.dram_tensor("out", [R, DM], F32, kind="ExternalOutput").ap()

    Exp = mybir.ActivationFunctionType.Exp
    Sigmoid = mybir.ActivationFunctionType.Sigmoid
    Op = mybir.AluOpType

    from contextlib import ExitStack
    with ExitStack() as ctx:
        acts = ctx.enter_context(tc.tile_pool(name="acts", bufs=1))
        work = ctx.enter_context(tc.tile_pool(name="work", bufs=2))
        sqp = ctx.enter_context(tc.tile_pool(name="sqp", bufs=1))
        psum = ctx.enter_context(tc.tile_pool(name="psum", bufs=2, space="PSUM"))
        dram = ctx.enter_context(tc.tile_pool(name="dram", bufs=1, space="DRAM"))

        # Dummy 4-byte AllReduce issued first: the first collective doorbell
        # pays a ~60us one-time channel setup; absorbing it here (concurrent
        # with the DMA ramp and K/V phase) makes the real AllReduce ~12us.
        # The warmup matmuls on zeros un-throttle the PE clock while the
        # first input chunks stream in; their result (0.0) feeds the dummy
        # collective so they are not dead code.
        warm = acts.tile([P, 640], F16, name="warm")
        nc.gpsimd.memset(warm[:], 0.0)
        pwu = psum.tile([P, NB], F32, name="pwu", tag="pk")
        for _ in range(14):
            nc.tensor.matmul(pwu[:], warm[:, 0:P], warm[:, P:P + NB],
                             start=True, stop=True)
        dummy = acts.tile([1, 1], F32, name="dummy")
        nc.vector.tensor_copy(dummy[:], pwu[0:1, 0:1])
        d_in = dram.tile([1, 1], F32, name="d_in")
        d_out = dram.tile([1, 1], F32, name="d_out", addr_space="Shared")
        nc.sync.dma_start(d_in[:], dummy[:])
        nc.gpsimd.collective_compute(
            "AllReduce", mybir.AluOpType.add,
            replica_groups=[list(range(N_CORES))],
            ins=[d_in.opt()], outs=[d_out.opt()],
        )

        def declare(name, free):
            return acts.tile([P, MC, free], F16, name=name)

        def load_chunk(t, ap_dram, mc):
            src = ap_dram.rearrange("(c p) f -> p c f", p=P)
            nc.sync.dma_start(t[:, mc, :], src[:, mc, :])

        def load(ap_dram, name, free):
            t = declare(name, free)
            nc.sync.dma_start(t[:], ap_dram.rearrange("(c p) f -> p c f", p=P))
            return t

        # K/V operands stream in per-mc so the first matmuls start early
        # (sync ring drains in order).
        kt = declare("kt", R)
        wk = declare("wk", DH)
        for mc in range(MC):
            load_chunk(wk, wkT, mc)
            load_chunk(kt, kT, mc)
        vt = declare("vt", R)
        wv = declare("wv", DH)
        for mc in range(MC):
            load_chunk(wv, wvT, mc)
            load_chunk(vt, vT, mc)
        qt = load(qT, "qt", R)
        wq = load(wqT, "wq", DH)
        wo = load(woT, "wo", DM)

        nd_all = acts.tile([P, HB, 8], F32, name="nd_all")

        # ---- K/V projections + exp + partial num/den ----
        for hb in range(HB):
            hs = slice(hb * P, (hb + 1) * P)
            pk = psum.tile([P, R], F32, name="pk", tag="pk")
            for mc in range(MC):
                for rb in range(RB):
                    nc.tensor.matmul(
                        pk[:, rb * NB:(rb + 1) * NB],
                        wk[:, mc, hs],
                        kt[:, mc, rb * NB:(rb + 1) * NB],
                        start=(mc == 0), stop=(mc == MC - 1),
                    )
            pv = psum.tile([P, R], F32, name="pv", tag="pv")
            for mc in range(MC):
                for rb in range(RB):
                    nc.tensor.matmul(
                        pv[:, rb * NB:(rb + 1) * NB],
                        wv[:, mc, hs],
                        vt[:, mc, rb * NB:(rb + 1) * NB],
                        start=(mc == 0), stop=(mc == MC - 1),
                    )
            # max over batch: copy + 3 maxes (DVE may read PSUM only once/op)
            mk = work.tile([P, TC], F32, name="mk")
            nc.vector.tensor_copy(mk[:], pk[:, 0 * TC:1 * TC])
            for b in range(1, B):
                nc.vector.tensor_tensor(
                    mk[:], mk[:], pk[:, b * TC:(b + 1) * TC], op=Op.max)
            ek = work.tile([P, R], F32, name="ek")
            ek3 = ek.rearrange("p (b t) -> p b t", t=TC)
            nc.vector.tensor_tensor(
                ek3, pk.rearrange("p (b t) -> p b t", t=TC),
                mk[:, None, :].to_broadcast((P, B, TC)), op=Op.subtract)
            # exp on ACT with fused per-b den accumulation (contiguous slices)
            for b in range(B):
                bs = slice(b * TC, (b + 1) * TC)
                nc.scalar.activation(
                    ek[:, bs], ek[:, bs], Exp,
                    accum_out=nd_all[:, hb, b:b + 1])
            ekv = work.tile([P, R], F32, name="ekv")
            nc.vector.tensor_tensor(ekv[:], ek[:], pv[:], op=Op.mult)
            nc.vector.tensor_reduce(
                nd_all[:, hb, B:2 * B], ekv.rearrange("p (b t) -> p b t", t=TC),
                axis=mybir.AxisListType.X, op=Op.add)

        # ---- AllReduce of partial num/den (32 KiB) ----
        red_in = dram.tile([P, HB * 8], F32, name="red_in")
        red_out = dram.tile([P, HB * 8], F32, name="red_out", addr_space="Shared")
        nc.sync.dma_start(red_in[:], nd_all[:])
        nc.gpsimd.collective_compute(
            "AllReduce", Op.add,
            replica_groups=[list(range(N_CORES))],
            ins=[red_in.opt()], outs=[red_out.opt()],
        )
        nd_sum = acts.tile([P, HB, 8], F32, name="nd_sum")
        nc.sync.dma_start(nd_sum[:], red_out[:])

        # ---- Q projection + sigmoid (overlaps the collective) ----
        sq = []
        for hb in range(HB):
            hs = slice(hb * P, (hb + 1) * P)
            pq = psum.tile([P, R], F32, name="pq", tag="pk")
            for mc in range(MC):
                for rb in range(RB):
                    nc.tensor.matmul(
                        pq[:, rb * NB:(rb + 1) * NB],
                        wq[:, mc, hs],
                        qt[:, mc, rb * NB:(rb + 1) * NB],
                        start=(mc == 0), stop=(mc == MC - 1),
                    )
            s = sqp.tile([P, R], F16, name=f"sq{hb}")
            nc.scalar.activation(s[:], pq[:], Sigmoid)
            sq.append(s)

        # ---- r = num/den, yt = sigmoid(Q) * r ----
        rden = acts.tile([P, HB, B], F32, name="rden")
        nc.vector.reciprocal(rden[:], nd_sum[:, :, 0:B])
        r_bf = acts.tile([P, HB, B], F16, name="r_bf")
        nc.vector.tensor_tensor(r_bf[:], nd_sum[:, :, B:2 * B], rden[:], op=Op.mult)

        # ---- output projection (yt = sig*r folded in just before first use
        # so the first O matmuls don't wait for the whole yt chain) ----
        for rblk in range(HB):
            rs = slice(rblk * P, (rblk + 1) * P)
            po = psum.tile([P, DM], F32, name="po", tag="pv")
            for hc in range(HB):
                if rblk == 0:
                    s3 = sq[hc].rearrange("p (b t) -> p b t", t=TC)
                    nc.vector.tensor_tensor(
                        s3, s3, r_bf[:, hc, :, None].to_broadcast((P, B, TC)),
                        op=Op.mult)
                for mb in range(MB):
                    nc.tensor.matmul(
                        po[:, mb * NB:(mb + 1) * NB],
                        sq[hc][:, rs],
                        wo[:, hc, mb * NB:(mb + 1) * NB],
                        start=(hc == 0), stop=(hc == HB - 1),
                    )
            ot = work.tile([P, DM], F32, name="ot")
            if rblk % 2 == 0:
                nc.vector.tensor_copy(ot[:], po[:])
            else:
                nc.scalar.copy(ot[:], po[:])
            nc.sync.dma_start(out[rs, :], ot[:])


def _build():
    global _GRAPH
    if _GRAPH is None:
        nc = bacc.Bacc("TRN2", target_bir_lowering=False, debug=False,
                       num_devices=N_CORES)
        with tile.TileContext(nc) as tc:
            _body(nc, tc)
        nc.compile()
        _GRAPH = nc
    return _GRAPH


def _shard_inputs(inputs):
    q = np.asarray(inputs["q"], np.float32)
    k = np.asarray(inputs["k"], np.float32)
    v = np.asarray(inputs["v"], np.float32)
    wqT = np.ascontiguousarray(np.asarray(inputs["Wq"], np.float32).T).astype(NPF16)
    wkT = np.ascontiguousarray(np.asarray(inputs["Wk"], np.float32).T).astype(NPF16)
    wvT = np.ascontiguousarray(np.asarray(inputs["Wv"], np.float32).T).astype(NPF16)
    woT = np.ascontiguousarray(np.asarray(inputs["Wo"], np.float32).T).astype(NPF16)

    def tslice(x, c):
        # (B, TC, DM) -> (DM, B, TC) -> (DM, R) with r = b*256 + t
        s = x[:, c * TC:(c + 1) * TC, :].transpose(2, 0, 1)
        return np.ascontiguousarray(s).reshape(DM, R).astype(NPF16)

    in_maps = []
    for c in range(N_CORES):
        in_maps.append({
            "qT": tslice(q, c),
            "kT": tslice(k, c),
            "vT": tslice(v, c),
            "wqT": wqT, "wkT": wkT, "wvT": wvT, "woT": woT,
        })
    return in_maps


def _unshard(outs):
    full = np.empty((B, T, DM), np.float32)
    for c in range(N_CORES):
        # out_c[r, m] with r = b*256 + t  ->  (b, t, m)
        full[:, c * TC:(c + 1) * TC, :] = outs[c].reshape(B, TC, DM)
    return full


def run(inputs, trace=False, trace_cores=None, **kw):
    nc = _build()
    in_maps = _shard_inputs(inputs)
    res = run_bass_kernel_spmd(
        nc, in_maps, list(range(N_CORES)),
        trace=trace, trace_cores=trace_cores, **kw)
    return _unshard([m["out"] for m in res.results]), res


def kernel(**inputs):
    out, _ = run(inputs)
    return out


# revision 13
# speedup vs baseline: 1.0482x; 1.0096x over previous
"""AFT-Full attention kernel for 8 TRN2 NeuronCores.

Math: the reference's exp_pos_bias = exp(pos_bias - max(pos_bias, axis=0)) is
identically 1.0 (the max is over a singleton dim), so the two (b,Ti,Tj,Dh)
einsums collapse to per-(b,h) sums over j:
    num[b,h] = sum_j exp(K-max_b K)[b,j,h] * V[b,j,h]
    den[b,h] = sum_j exp(K-max_b K)[b,j,h]
    out = (sigmoid(Q) * num/den) @ Wo.T

Sharding: sequence-parallel over T (256 positions per core, all 4 batches),
weights replicated.  Per-core row index r = b*256 + t_local (b-major) so the
per-(b,h) sums over t are contiguous innermost reduces.  One 32 KiB AllReduce
combines the per-core partial num/den; a 4-byte dummy AllReduce issued at
kernel start absorbs the ~60us one-time collective channel setup so the real
one (~12us) hides behind the Q projection.
"""

import numpy as np
import ml_dtypes

import concourse.bass as bass
import concourse.mybir as mybir
import concourse.tile as tile
from concourse import bacc
from concourse.bass_utils import run_bass_kernel_spmd

B, T, DM, DH = 4, 2048, 1024, 1024
N_CORES = 8
TC = T // N_CORES          # 256 sequence positions per core
R = B * TC                 # 1024 rows per core, r = b*256 + t
P = 128
MC = DM // P               # 8 contraction chunks (d_model)
HB = DH // P               # 8 hidden blocks
NB = 512                   # matmul moving free dim
RB = R // NB               # 2 row blocks
MB = DM // NB              # 2 output-model blocks

F16 = mybir.dt.float16
F32 = mybir.dt.float32
NPF16 = np.float16

_GRAPH = None


def _body(nc, tc):
    qT = nc.dram_tensor("qT", [DM, R], F16, kind="ExternalInput").ap()
    kT = nc.dram_tensor("kT", [DM, R], F16, kind="ExternalInput").ap()
    vT = nc.dram_tensor("vT", [DM, R], F16, kind="ExternalInput").ap()
    wqT = nc.dram_tensor("wqT", [DM, DH], F16, kind="ExternalInput").ap()
    wkT = nc.dram_tensor("wkT", [DM, DH], F16, kind="ExternalInput").ap()
    wvT = nc.dram_tensor("wvT", [DM, DH], F16, kind="ExternalInput").ap()
    woT = nc.dram_tensor("woT", [DH, DM], F16, kind="ExternalInput").ap()
    out = nc.dram_tensor("out", [R, DM], F32, kind="ExternalOutput").ap()

    Exp = mybir.ActivationFunctionType.Exp
    Sigmoid = mybir.ActivationFunctionType.Sigmoid
    Op = mybir.AluOpType

    from contextlib import ExitStack
    with ExitStack() as ctx:
        acts = ctx.enter_context(tc.tile_pool(name="acts", bufs=1))
        work = ctx.enter_context(tc.tile_pool(name="work", bufs=2))
        sqp = ctx.enter_context(tc.tile_pool(name="sqp", bufs=1))
        psum = ctx.enter_context(tc.tile_pool(name="psum", bufs=2, space="PSUM"))
        dram = ctx.enter_context(tc.tile_pool(name="dram", bufs=1, space="DRAM"))

        # Dummy 4-byte AllReduce issued first: the first collective doorbell
        # pays a ~60us one-time channel setup; absorbing it here (concurrent
        # with the DMA ramp and K/V phase) makes the real AllReduce ~12us.
        # The warmup matmuls on zeros un-throttle the PE clock while the
        # first input chunks stream in; their result (0.0) feeds the dummy
        # collective so they are not dead code.
        warm = acts.tile([P, 640], F16, name="warm")
        nc.gpsimd.memset(warm[:], 0.0)
        pwu = psum.tile([P, NB], F32, name="pwu", tag="pk")
        for _ in range(14):
            nc.tensor.matmul(pwu[:], warm[:, 0:P], warm[:, P:P + NB],
                             start=True, stop=True)
        dummy = acts.tile([1, 1], F32, name="dummy")
        nc.vector.tensor_copy(dummy[:], pwu[0:1, 0:1])
        d_in = dram.tile([1, 1], F32, name="d_in")
        d_out = dram.tile([1, 1], F32, name="d_out", addr_space="Shared")
        nc.sync.dma_start(d_in[:], dummy[:])
        nc.gpsimd.collective_compute(
            "AllReduce", mybir.AluOpType.add,
            replica_groups=[list(range(N_CORES))],
            ins=[d_in.opt()], outs=[d_out.opt()],
        )

        def declare(name, free):
            return acts.tile([P, MC, free], F16, name=name)

        def load_chunk(t, ap_dram, mc):
            src = ap_dram.rearrange("(c p) f -> p c f", p=P)
            nc.sync.dma_start(t[:, mc, :], src[:, mc, :])

        def load(ap_dram, name, free):
            t = declare(name, free)
            nc.sync.dma_start(t[:], ap_dram.rearrange("(c p) f -> p c f", p=P))
            return t

        # K/V operands stream in per-mc so the first matmuls start early
        # (sync ring drains in order).
        kt = declare("kt", R)
        wk = declare("wk", DH)
        for mc in range(MC):
            load_chunk(wk, wkT, mc)
            load_chunk(kt, kT, mc)
        vt = declare("vt", R)
        wv = declare("wv", DH)
        for mc in range(MC):
            load_chunk(wv, wvT, mc)
            load_chunk(vt, vT, mc)
        qt = load(qT, "qt", R)
        wq = load(wqT, "wq", DH)
        wo = load(woT, "wo", DM)

        nd_all = acts.tile([P, HB, 8], F32, name="nd_all")

        # ---- K/V projections + exp + partial num/den ----
        # K runs one hb ahead of V: kt/wk arrive first, and the PE clock is
        # un-throttled only for the first ~24us, so front-load K matmuls.
        def kproj(hb):
            hs = slice(hb * P, (hb + 1) * P)
            pk = psum.tile([P, R], F32, name="pk", tag="pk")
            for mc in range(MC):
                for rb in range(RB):
                    nc.tensor.matmul(
                        pk[:, rb * NB:(rb + 1) * NB],
                        wk[:, mc, hs],
                        kt[:, mc, rb * NB:(rb + 1) * NB],
                        start=(mc == 0), stop=(mc == MC - 1),
                    )
            return pk

        pk_next = kproj(0)
        for hb in range(HB):
            hs = slice(hb * P, (hb + 1) * P)
            pk = pk_next
            if hb + 1 < HB:
                pk_next = kproj(hb + 1)
            pv = psum.tile([P, R], F32, name="pv", tag="pv")
            for mc in range(MC):
                for rb in range(RB):
                    nc.tensor.matmul(
                        pv[:, rb * NB:(rb + 1) * NB],
                        wv[:, mc, hs],
                        vt[:, mc, rb * NB:(rb + 1) * NB],
                        start=(mc == 0), stop=(mc == MC - 1),
                    )
            # max over batch: copy + 3 maxes (DVE may read PSUM only once/op)
            mk = work.tile([P, TC], F32, name="mk")
            nc.vector.tensor_copy(mk[:], pk[:, 0 * TC:1 * TC])
            for b in range(1, B):
                nc.vector.tensor_tensor(
                    mk[:], mk[:], pk[:, b * TC:(b + 1) * TC], op=Op.max)
            ek = work.tile([P, R], F32, name="ek")
            ek3 = ek.rearrange("p (b t) -> p b t", t=TC)
            nc.vector.tensor_tensor(
                ek3, pk.rearrange("p (b t) -> p b t", t=TC),
                mk[:, None, :].to_broadcast((P, B, TC)), op=Op.subtract)
            # exp on ACT with fused per-b den accumulation (contiguous slices)
            for b in range(B):
                bs = slice(b * TC, (b + 1) * TC)
                nc.scalar.activation(
                    ek[:, bs], ek[:, bs], Exp,
                    accum_out=nd_all[:, hb, b:b + 1])
            ekv = work.tile([P, R], F32, name="ekv")
            nc.vector.tensor_tensor(ekv[:], ek[:], pv[:], op=Op.mult)
            nc.vector.tensor_reduce(
                nd_all[:, hb, B:2 * B], ekv.rearrange("p (b t) -> p b t", t=TC),
                axis=mybir.AxisListType.X, op=Op.add)

        # ---- AllReduce of partial num/den (32 KiB) ----
        red_in = dram.tile([P, HB * 8], F32, name="red_in")
        red_out = dram.tile([P, HB * 8], F32, name="red_out", addr_space="Shared")
        nc.sync.dma_start(red_in[:], nd_all[:])
        nc.gpsimd.collective_compute(
            "AllReduce", Op.add,
            replica_groups=[list(range(N_CORES))],
            ins=[red_in.opt()], outs=[red_out.opt()],
        )
        nd_sum = acts.tile([P, HB, 8], F32, name="nd_sum")
        nc.sync.dma_start(nd_sum[:], red_out[:])

        # ---- Q projection + sigmoid (overlaps the collective) ----
        sq = []
        for hb in range(HB):
            hs = slice(hb * P, (hb + 1) * P)
            pq = psum.tile([P, R], F32, name="pq", tag="pk")
            for mc in range(MC):
                for rb in range(RB):
                    nc.tensor.matmul(
                        pq[:, rb * NB:(rb + 1) * NB],
                        wq[:, mc, hs],
                        qt[:, mc, rb * NB:(rb + 1) * NB],
                        start=(mc == 0), stop=(mc == MC - 1),
                    )
            s = sqp.tile([P, R], F16, name=f"sq{hb}")
            nc.scalar.activation(s[:], pq[:], Sigmoid)
            sq.append(s)

        # ---- r = num/den, yt = sigmoid(Q) * r ----
        rden = acts.tile([P, HB, B], F32, name="rden")
        nc.vector.reciprocal(rden[:], nd_sum[:, :, 0:B])
        r_bf = acts.tile([P, HB, B], F16, name="r_bf")
        nc.vector.tensor_tensor(r_bf[:], nd_sum[:, :, B:2 * B], rden[:], op=Op.mult)

        # ---- output projection (yt = sig*r folded in just before first use
        # so the first O matmuls don't wait for the whole yt chain) ----
        for rblk in range(HB):
            rs = slice(rblk * P, (rblk + 1) * P)
            po = psum.tile([P, DM], F32, name="po", tag="pv")
            for hc in range(HB):
                if rblk == 0:
                    s3 = sq[hc].rearrange("p (b t) -> p b t", t=TC)
                    nc.vector.tensor_tensor(
                        s3, s3, r_bf[:, hc, :, None].to_broadcast((P, B, TC)),
                        op=Op.mult)
                for mb in range(MB):
                    nc.tensor.matmul(
                        po[:, mb * NB:(mb + 1) * NB],
                        sq[hc][:, rs],
                        wo[:, hc, mb * NB:(mb + 1) * NB],
                        start=(hc == 0), stop=(hc == HB - 1),
                    )
            ot = work.tile([P, DM], F32, name="ot")
            if rblk % 2 == 0:
                nc.vector.tensor_copy(ot[:], po[:])
            else:
                nc.scalar.copy(ot[:], po[:])
            nc.sync.dma_start(out[rs, :], ot[:])


def _build():
    global _GRAPH
    if _GRAPH is None:
        nc = bacc.Bacc("TRN2", target_bir_lowering=False, debug=False,
                       num_devices=N_CORES)
        with tile.TileContext(nc) as tc:
            _body(nc, tc)
        nc.compile()
        _GRAPH = nc
    return _GRAPH


def _shard_inputs(inputs):
    q = np.asarray(inputs["q"], np.float32)
    k = np.asarray(inputs["k"], np.float32)
    v = np.asarray(inputs["v"], np.float32)
    wqT = np.ascontiguousarray(np.asarray(inputs["Wq"], np.float32).T).astype(NPF16)
    wkT = np.ascontiguousarray(np.asarray(inputs["Wk"], np.float32).T).astype(NPF16)
    wvT = np.ascontiguousarray(np.asarray(inputs["Wv"], np.float32).T).astype(NPF16)
    woT = np.ascontiguousarray(np.asarray(inputs["Wo"], np.float32).T).astype(NPF16)

    def tslice(x, c):
        # (B, TC, DM) -> (DM, B, TC) -> (DM, R) with r = b*256 + t
        s = x[:, c * TC:(c + 1) * TC, :].transpose(2, 0, 1)
        return np.ascontiguousarray(s).reshape(DM, R).astype(NPF16)

    in_maps = []
    for c in range(N_CORES):
        in_maps.append({
            "qT": tslice(q, c),
            "kT": tslice(k, c),
            "vT": tslice(v, c),
            "wqT": wqT, "wkT": wkT, "wvT": wvT, "woT": woT,
        })
    return in_maps


def _unshard(outs):
    full = np.empty((B, T, DM), np.float32)
    for c in range(N_CORES):
        # out_c[r, m] with r = b*256 + t  ->  (b, t, m)
        full[:, c * TC:(c + 1) * TC, :] = outs[c].reshape(B, TC, DM)
    return full


def run(inputs, trace=False, trace_cores=None, **kw):
    nc = _build()
    in_maps = _shard_inputs(inputs)
    res = run_bass_kernel_spmd(
        nc, in_maps, list(range(N_CORES)),
        trace=trace, trace_cores=trace_cores, **kw)
    return _unshard([m["out"] for m in res.results]), res


def kernel(**inputs):
    out, _ = run(inputs)
    return out
